# revision 17
# baseline (speedup 1.0000x reference)
"""ConvNeXt block (dwconv7 -> LN -> pwconv1 -> GELU -> GRN -> pwconv2 -> residual)
for Trainium2, batch-parallel across 8 NeuronCores (2 batches per core).

Self-contained: hardcodes shapes B=16, C=512, T=2048, I=1536, K=7.

v3 design notes:
  - everything f16 on-chip (x shipped f16 from host): DVE 2x/4x perf modes,
    half DMA, h kept fully in SBUF (no DRAM round-trip).
  - conv = per-tap tensor_scalar products (4x mode) + tensor_tensor adds
    (2x mode), emitted slice-by-slice interleaved with the per-t-tile
    LN/mm1 pipelines so mm1 starts ~16us in, not after the full conv.
  - LN folded as ysc = y*Abc + Bbc (A=1/sqrt(var+eps), B=-mu*A broadcast
    via K=1 matmuls); no w1s rank-1 matmuls on PE.
  - batch 1's conv is emitted before batch 0's mm2 (runs on DVE during the
    mm2 window); batch 1's t0 stats chain is emitted in the middle of the
    mm2 stream so mm1(b1) starts right after mm2(b0).
  - PE warmup dummies at t=0 to exit the cold 1.2GHz p-state.

Math folding (host-side, weight-sized only):
  LN:  y_ln = (y - mu_t) * A_t * ln_g + ln_b
  mm1: h_pre = w1p @ ((y - mu)*A) + b1p, w1p = w1*ln_g, b1p = b1 + w1@ln_b
  GRN: h' = h * ss[i] + grn_b[i],  ss = 1 + grn_g * gx * d,
       gx = sqrt(sum_t h^2), d = 1/(mean_i gx + eps)
  mm2: out = (w2 * ss) @ h + (b2 + w2 @ grn_b) + residual
"""
import sys

sys.path.insert(0, "/opt/trn_rl_repo")

import numpy as np
import concourse.bacc as bacc
import concourse.tile as tile
from concourse import mybir
from concourse.bass_utils import run_bass_kernel_spmd

F32 = mybir.dt.float32
F32R = mybir.dt.float32r
F16 = mybir.dt.float16
AF = mybir.ActivationFunctionType
OP = mybir.AluOpType

B, C, T, I, K = 16, 512, 2048, 1536, 7
NCORES = 8
BPC = B // NCORES          # batches per core
CC = C // 128              # 4 c-chunks
IC = I // 128              # 12 i-chunks
TC = T // 512              # 4 t-chunks
TN = 512                   # matmul free-dim tile
LN_EPS = 1e-6
GRN_EPS = 1e-6

_CACHE = {}


def _build(trace_sim=False, reps=1):
    nc = bacc.Bacc("TRN2", target_bir_lowering=False, debug=False,
                   num_devices=NCORES)
    dram = {}

    def din(name, shape, dt=F32):
        dram[name] = nc.dram_tensor(name, shape, dt, kind="ExternalInput").ap()
        return dram[name]

    x_d = din("x", [BPC, C, T], F16)                 # per-core batches (f16!)
    w1pT_d = din("w1pT", [C, I], F16)                # (w1*ln_g).T  [c, i]
    b1p_d = din("b1p", [128, IC])                    # col-chunked b1p
    w2T_d = din("w2T", [I, C], F16)                  # w2.T  [i, c]
    b2p_d = din("b2p", [128, CC])                    # col-chunked b2 + w2@grn_b
    grng_d = din("grng", [128, IC])                  # col-chunked grn_g
    dww_d = din("dww", [128, CC, K])                 # depthwise taps per c-chunk
    dwb_d = din("dwb", [128, CC])                    # depthwise bias per c-chunk
    out_d = nc.dram_tensor("out", [BPC, C, T], F32, kind="ExternalOutput").ap()

    with tile.TileContext(nc, trace_sim=trace_sim) as tc:
        for _ in range(reps):
            _kernel_body(nc, tc, x_d, w1pT_d, b1p_d, w2T_d, b2p_d,
                         grng_d, dww_d, dwb_d, out_d)
    nc.compile()
    return nc


def _kernel_body(nc, tc, x_d, w1pT_d, b1p_d, w2T_d, b2p_d,
                 grng_d, dww_d, dwb_d, out_d):
    from contextlib import ExitStack
    ctx = ExitStack()
    with ctx:
        ctx.enter_context(nc.allow_low_precision(
            reason="f16 on-chip compute is intentional; tolerance is 2e-2"))
        singles = ctx.enter_context(tc.tile_pool(name="singles", bufs=1))
        xp = ctx.enter_context(tc.tile_pool(name="xp", bufs=2))
        yp = ctx.enter_context(tc.tile_pool(name="yp", bufs=2))
        pp = ctx.enter_context(tc.tile_pool(name="pp", bufs=2))
        ysqp = ctx.enter_context(tc.tile_pool(name="ysqp", bufs=4))
        rowp = ctx.enter_context(tc.tile_pool(name="rowp", bufs=4))
        abp = ctx.enter_context(tc.tile_pool(name="abp", bufs=2))
        yscp = ctx.enter_context(tc.tile_pool(name="yscp", bufs=2))
        hp = ctx.enter_context(tc.tile_pool(name="hp", bufs=4))
        sqp = ctx.enter_context(tc.tile_pool(name="sqp", bufs=2))
        gxp = ctx.enter_context(tc.tile_pool(name="gxp", bufs=2))
        w2p = ctx.enter_context(tc.tile_pool(name="w2p", bufs=1))
        w2sp = ctx.enter_context(tc.tile_pool(name="w2sp", bufs=1))
        op_ = ctx.enter_context(tc.tile_pool(name="op", bufs=2))
        # PSUM pools: 2 ph + 2 po + 2 stats + 2 (one [128,1024] AB) = 8 banks
        mmps = ctx.enter_context(tc.tile_pool(name="mmps", bufs=2, space="PSUM"))
        pops = ctx.enter_context(tc.tile_pool(name="pops", bufs=2, space="PSUM"))
        smps = ctx.enter_context(tc.tile_pool(name="smps", bufs=2, space="PSUM"))
        abps = ctx.enter_context(tc.tile_pool(name="abps", bufs=2, space="PSUM"))

        # ---- tiny constants (no DMA deps) ----
        onesr_f = singles.tile([1, 128], F32)
        nc.vector.memset(onesr_f[:], 1.0)
        ones_row = singles.tile([1, 128], F16)    # bcast lhsT (K=1, M=128)
        nc.vector.tensor_copy(ones_row[:], onesr_f[:])
        invc_f = singles.tile([128, 1], F32)
        nc.vector.memset(invc_f[:], 1.0 / C)
        ones_col = singles.tile([128, 1], F16)    # stats lhsT: gives mean directly
        nc.vector.tensor_copy(ones_col[:], invc_f[:])
        ones1_f = singles.tile([128, 1], F32)
        nc.vector.memset(ones1_f[:], 1.0)
        ones_col1 = singles.tile([128, 1], F32R)  # gsum lhsT (K=128, M=1)
        nc.vector.tensor_copy(ones_col1[:], ones1_f[:])
        eps_ln = singles.tile([1, 1], F32)
        nc.vector.memset(eps_ln[:], LN_EPS)
        wrow = singles.tile([1, TN], F16)
        nc.vector.memset(wrow[:], 0.0)

        # ---- PE warmup: exit cold p-state while x DMA + conv run ----
        for _ in range(24):
            wps = mmps.tile([128, TN], F32, tag="mm")
            nc.tensor.matmul(wps[:], ones_row[:], wrow[:], start=True, stop=True)

        # ---- constants ----
        dww = singles.tile([128, CC, K], F32)
        nc.gpsimd.dma_start(dww[:], dww_d)
        dwb = singles.tile([128, CC], F32)
        nc.gpsimd.dma_start(dwb[:], dwb_d)
        b1p = singles.tile([128, IC], F32)
        nc.gpsimd.dma_start(b1p[:], b1p_d)
        b2p = singles.tile([128, CC], F32)
        nc.gpsimd.dma_start(b2p[:], b2p_d)
        grng = singles.tile([128, IC], F32)
        nc.gpsimd.dma_start(grng[:], grng_d)
        w1pT = singles.tile([128, CC, I], F16)
        nc.sync.dma_start(w1pT[:], w1pT_d.rearrange("(cc p) i -> p cc i", p=128))
        w2t = w2p.tile([128, IC, C], F16)
        nc.sync.dma_start(w2t[:], w2T_d.rearrange("(ic p) c -> p ic c", p=128))

        xv = x_d.rearrange("b (cc p) t -> b p cc t", p=128)

        # ---- per-batch persistent tiles ----
        x_tiles = []
        for b in range(BPC):
            x_t = xp.tile([128, CC, T], F16, tag="x")
            for ci in range(CC):
                nc.gpsimd.dma_start(x_t[:, ci, :], xv[b, :, ci, :])
            x_tiles.append(x_t)

        y_tiles = {}
        gxparts = {}
        h_tiles = {}

        def emit_conv_slice(b, a0, a1, pool_cc=(), ccs=None):
            """Depthwise conv over [a0, a1): TSP products (4x) + TT adds (2x).
            Chunks in pool_cc run as stt chains on GPSIMD (startup balance)."""
            x_t = x_tiles[b]
            y_t = y_tiles[b]
            n = a1 - a0
            for ci in (range(CC) if ccs is None else ccs):
                acc = y_t[:, ci, a0:a1]
                eng = nc.gpsimd if ci in pool_cc else nc.vector
                if ci in pool_cc:
                    # Pool rejects the two-scalar-op TSP form; add bias last
                    eng.tensor_scalar(acc, x_t[:, ci, a0:a1],
                                      dww[:, ci, 3:4], None, OP.mult)
                else:
                    eng.tensor_scalar(acc, x_t[:, ci, a0:a1],
                                      dww[:, ci, 3:4], dwb[:, ci:ci + 1],
                                      OP.mult, OP.add)
                for k in (0, 1, 2, 4, 5, 6):
                    d = k - 3
                    lo = max(a0, -d)
                    hi = min(a1, T - d)
                    m = hi - lo
                    if ci in pool_cc:
                        eng.scalar_tensor_tensor(
                            acc[:, lo - a0:hi - a0],
                            x_t[:, ci, lo + d:hi + d],
                            dww[:, ci, k:k + 1],
                            acc[:, lo - a0:hi - a0], OP.mult, OP.add)
                        if k == 6:
                            eng.tensor_scalar(acc, acc, dwb[:, ci:ci + 1],
                                              None, OP.add)
                        continue
                    p_k = pp.tile([128, n], F16, tag="p",
                                  padded_shape=[128, 1024])
                    nc.vector.tensor_scalar(p_k[:, 0:m],
                                            x_t[:, ci, lo + d:hi + d],
                                            dww[:, ci, k:k + 1], None,
                                            OP.mult)
                    nc.vector.tensor_add(acc[:, lo - a0:hi - a0],
                                         acc[:, lo - a0:hi - a0],
                                         p_k[:, 0:m])

        ysq_tiles = {}

        def emit_ysq_cc(b, t, ci):
            y_t = y_tiles[b]
            ts_ = slice(t * TN, (t + 1) * TN)
            ysq = ysqp.tile([128, TN], F16, tag="ysq")
            nc.scalar.activation(ysq[:], y_t[:, ci, ts_], AF.Square)
            ysq_tiles.setdefault((b, t), {})[ci] = ysq

        def emit_ysq(b, t):
            for ci in range(CC):
                emit_ysq_cc(b, t, ci)

        def emit_tile_stats(b, t):
            """LN stats + A/B rows + broadcast + ysc for t-tile. Returns ysc.
            High priority: this chain feeds mm1 and must preempt conv work."""
            with tc.high_priority(offset=400):
                return _emit_tile_stats(b, t)

        def _emit_tile_stats(b, t):
            y_t = y_tiles[b]
            ts_ = slice(t * TN, (t + 1) * TN)
            ysq = ysq_tiles.pop((b, t))
            mu_ps = smps.tile([1, TN], F32, tag="sm")
            ey2_ps = smps.tile([1, TN], F32, tag="sm")
            for ci in range(CC):
                nc.tensor.matmul(mu_ps[:], ones_col[:], y_t[:, ci, ts_],
                                 start=(ci == 0), stop=(ci == CC - 1))
                nc.tensor.matmul(ey2_ps[:], ones_col[:], ysq[ci][:],
                                 start=(ci == 0), stop=(ci == CC - 1))
            mu_sb = rowp.tile([1, TN], F32, tag="r")
            nc.vector.tensor_copy(mu_sb[:], mu_ps[:])
            msq = rowp.tile([1, TN], F32, tag="r")
            nc.vector.tensor_mul(msq[:], mu_sb[:], mu_sb[:])
            var = rowp.tile([1, TN], F32, tag="r")
            nc.vector.tensor_sub(var[:], ey2_ps[:], msq[:])
            stdv = rowp.tile([1, TN], F32, tag="r")
            nc.scalar.activation(stdv[:], var[:], AF.Sqrt, bias=eps_ln[:])
            ab_row = rowp.tile([1, 2 * TN], F16, tag="ab")
            nc.vector.reciprocal(ab_row[:, 0:TN], stdv[:])
            # B = -mu * A
            nc.vector.scalar_tensor_tensor(ab_row[:, TN:2 * TN], mu_sb[:], -1.0,
                                           ab_row[:, 0:TN], OP.mult, OP.mult)
            # broadcast A,B across partitions (K=1 matmuls)
            a_ps = abps.tile([128, TN], F32, tag="abps")
            b_ps = abps.tile([128, TN], F32, tag="abps")
            nc.tensor.matmul(a_ps[:], ones_row[:], ab_row[:, 0:TN],
                             start=True, stop=True)
            nc.tensor.matmul(b_ps[:], ones_row[:],
                             ab_row[:, TN:2 * TN], start=True, stop=True)
            ab_sb = abp.tile([128, 2 * TN], F16, tag="absb")
            nc.scalar.activation(ab_sb[:, 0:TN], a_ps[:], AF.Identity)
            nc.scalar.activation(ab_sb[:, TN:2 * TN], b_ps[:], AF.Identity)
            # ysc = y*A + B
            ysc = yscp.tile([128, CC, TN], F16, tag="ysc")
            for ci in range(CC):
                nc.vector.tensor_mul(ysc[:, ci, :], y_t[:, ci, ts_],
                                     ab_sb[:, 0:TN])
                nc.vector.tensor_add(ysc[:, ci, :], ysc[:, ci, :],
                                     ab_sb[:, TN:2 * TN])
            return ysc

        def emit_tile_mm1(b, t, ysc):
            gxpart = gxparts[b]
            h_t = hp.tile([128, IC, TN], F16, tag="h")
            h_tiles[(b, t)] = h_t
            for ii in range(IC):
                ph = mmps.tile([128, TN], F32, tag="mm")
                isl = slice(ii * 128, (ii + 1) * 128)
                for ci in range(CC):
                    nc.tensor.matmul(ph[:], w1pT[:, ci, isl],
                                     ysc[:, ci, :],
                                     start=(ci == 0), stop=(ci == CC - 1))
                nc.scalar.activation(h_t[:, ii, :], ph[:], AF.Gelu,
                                     bias=b1p[:, ii:ii + 1])
                sq = sqp.tile([128, TN], F16, tag="sq")
                if ii % 2 == 0:
                    nc.vector.scalar_tensor_tensor(
                        sq[:], h_t[:, ii, :], 1.0, h_t[:, ii, :],
                        OP.bypass, OP.mult,
                        accum_out=gxpart[:, ii, t:t + 1])
                else:
                    nc.scalar.activation(
                        sq[:], h_t[:, ii, :], AF.Square,
                        accum_out=gxpart[:, ii, t:t + 1])

        def emit_grn(b):
            """GRN scale factors -> scaled w2 copy."""
            gxpart = gxparts[b]
            gxsq = gxp.tile([128, IC], F32, tag="gx2")
            nc.vector.tensor_reduce(gxsq[:], gxpart[:],
                                    axis=mybir.AxisListType.X, op=OP.add)
            gx = gxp.tile([128, IC], F32R, tag="gx2")
            nc.scalar.activation(gx[:], gxsq[:], AF.Sqrt)
            gsum = smps.tile([1, IC], F32, tag="sm")
            nc.tensor.matmul(gsum[:], ones_col1[:], gx[:], start=True, stop=True)
            gtot = gxp.tile([1, 1], F32, tag="gx3")
            nc.vector.tensor_reduce(gtot[:], gsum[:], axis=mybir.AxisListType.X,
                                    op=OP.add)
            dinv = gxp.tile([1, 1], F32, tag="gx3")
            nc.vector.tensor_scalar(dinv[:], gtot[:], 1.0 / I, GRN_EPS,
                                    OP.mult, OP.add)
            d_row = gxp.tile([1, 1], F32R, tag="gx3")
            nc.vector.reciprocal(d_row[:], dinv[:])
            dbc = gxp.tile([128, 1], F32, tag="gx4")
            nc.gpsimd.partition_broadcast(dbc[:], d_row[:].bitcast(F32))
            ss = gxp.tile([128, IC], F32, tag="gx4")
            nc.vector.scalar_tensor_tensor(ss[:], gx[:].bitcast(F32), dbc[:],
                                           grng[:], OP.mult, OP.mult)
            nc.vector.tensor_scalar(ss[:], ss[:], 1.0, None, OP.add)
            w2sc = w2sp.tile([128, IC, C], F16, tag="w2sc")
            for ci in range(CC):
                csl = slice(ci * 128, (ci + 1) * 128)
                for ii in range(IC):
                    nc.vector.tensor_scalar(w2sc[:, ii, csl], w2t[:, ii, csl],
                                            ss[:, ii:ii + 1], None, OP.mult)
            return w2sc

        def emit_mm2(b, w2sc, prefetch=None):
            """mm2 + bias + residual + store. prefetch() emitted mid-stream."""
            x_t = x_tiles[b]
            group = 0
            for t in range(TC):
                ts_ = slice(t * TN, (t + 1) * TN)
                h_t = h_tiles[(b, t)]
                for ci in range(CC):
                    csl = slice(ci * 128, (ci + 1) * 128)
                    po = pops.tile([128, TN], F32, tag="po")
                    for ii in range(IC):
                        nc.tensor.matmul(po[:], w2sc[:, ii, csl],
                                         h_t[:, ii, :],
                                         start=(ii == 0), stop=(ii == IC - 1))
                    with tc.high_priority(offset=250):
                        o_sb = op_.tile([128, TN], F32)
                        nc.scalar.activation(o_sb[:], po[:], AF.Identity,
                                             bias=b2p[:, ci:ci + 1])
                        nc.gpsimd.tensor_add(o_sb[:], o_sb[:], x_t[:, ci, ts_])
                        nc.gpsimd.dma_start(
                            out_d[b, ci * 128:(ci + 1) * 128, ts_], o_sb[:])
                    group += 1
                    if group == 10 and prefetch is not None:
                        prefetch()

        # ================= schedule =================
        # stats chains run one t-tile ahead of mm1 so the LN row-math /
        # broadcast latency hides under the previous tile's matmuls; ysq
        # ops are emitted right after conv so they don't queue behind GELUs.
        CONV_SLICES = [(0, 512), (512, 1024), (1024, 2048)]
        prefetched_ysc = {}

        for b in range(BPC):
            if b not in y_tiles:
                y_tiles[b] = yp.tile([128, CC, T], F16, tag="y", name=f"y{b}")
                gxparts[b] = gxp.tile([128, IC, TC], F32, tag="gxpart",
                                      name=f"gxp{b}")
            if b in prefetched_ysc:
                ysc_prev = prefetched_ysc.pop(b)
            else:
                for ci in range(CC):
                    emit_conv_slice(b, *CONV_SLICES[0], ccs=(ci,))
                    emit_ysq_cc(b, 0, ci)
                ysc_prev = emit_tile_stats(b, 0)
                emit_conv_slice(b, *CONV_SLICES[1])
                emit_ysq(b, 1)
                emit_conv_slice(b, *CONV_SLICES[2])
                emit_ysq(b, 2)
                emit_ysq(b, 3)
            for t in range(TC):
                ysc_next = emit_tile_stats(b, t + 1) if t + 1 < TC else None
                emit_tile_mm1(b, t, ysc_prev)
                ysc_prev = ysc_next
            w2sc = emit_grn(b)
            prefetch = None
            if b + 1 < BPC:
                nb = b + 1
                y_tiles[nb] = yp.tile([128, CC, T], F16, tag="y", name=f"y{nb}")
                gxparts[nb] = gxp.tile([128, IC, TC], F32, tag="gxpart",
                                       name=f"gxp{nb}")
                for i, slc in enumerate(CONV_SLICES):
                    emit_conv_slice(nb, *slc)
                    emit_ysq(nb, min(i, 2))
                emit_ysq(nb, 3)

                def prefetch(nb=nb):
                    prefetched_ysc[nb] = emit_tile_stats(nb, 0)
            emit_mm2(b, w2sc, prefetch=prefetch)


def _host_prep(inputs):
    w1 = inputs["w1"].astype(np.float64)
    ln_g = inputs["ln_g"].astype(np.float64)
    ln_b = inputs["ln_b"].astype(np.float64)
    w2 = inputs["w2"].astype(np.float64)
    w1p = w1 * ln_g[None, :]                         # [I, C]
    prep = {
        "w1pT": np.ascontiguousarray(w1p.T).astype(np.float16),
        "b1p": (inputs["b1"].astype(np.float64) + w1 @ ln_b)
               .astype(np.float32).reshape(IC, 128).T.copy(),
        "w2T": np.ascontiguousarray(w2.T).astype(np.float16),
        "b2p": (inputs["b2"].astype(np.float64)
                + w2 @ inputs["grn_b"].astype(np.float64))
               .astype(np.float32).reshape(CC, 128).T.copy(),
        "grng": inputs["grn_g"].reshape(IC, 128).T.copy().astype(np.float32),
        "dww": inputs["dw_w"].reshape(C, K).reshape(CC, 128, K)
               .transpose(1, 0, 2).copy().astype(np.float32),
        "dwb": inputs["dw_b"].reshape(CC, 128).T.copy().astype(np.float32),
    }
    return prep


def run(inputs, trace=False, **kw):
    if "nc" not in _CACHE:
        _CACHE["nc"] = _build()
    nc = _CACHE["nc"]
    prep = _host_prep(inputs)
    x = np.asarray(inputs["x"], dtype=np.float32).astype(np.float16)
    in_maps = []
    for c in range(NCORES):
        m = dict(prep)
        m["x"] = np.ascontiguousarray(x[c * BPC:(c + 1) * BPC])
        in_maps.append(m)
    res = run_bass_kernel_spmd(nc, in_maps, core_ids=list(range(NCORES)),
                               trace=trace, **kw)
    out = np.concatenate([r["out"] for r in res.results], axis=0)
    return out, res


def kernel(**inputs):
    out, _ = run(inputs)
    return out


# revision 18
# speedup vs baseline: 2.6020x; 2.6020x over previous
"""ConvNeXt block (dwconv7 -> LN -> pwconv1 -> GELU -> GRN -> pwconv2 -> residual)
for Trainium2, batch-parallel across 8 NeuronCores (2 batches per core).

Self-contained: hardcodes shapes B=16, C=512, T=2048, I=1536, K=7.

v3 design notes:
  - everything f16 on-chip (x shipped f16 from host): DVE 2x/4x perf modes,
    half DMA, h kept fully in SBUF (no DRAM round-trip).
  - conv = per-tap tensor_scalar products (4x mode) + tensor_tensor adds
    (2x mode), emitted slice-by-slice interleaved with the per-t-tile
    LN/mm1 pipelines so mm1 starts ~16us in, not after the full conv.
  - LN folded as ysc = y*Abc + Bbc (A=1/sqrt(var+eps), B=-mu*A broadcast
    via K=1 matmuls); no w1s rank-1 matmuls on PE.
  - batch 1's conv is emitted before batch 0's mm2 (runs on DVE during the
    mm2 window); batch 1's t0 stats chain is emitted in the middle of the
    mm2 stream so mm1(b1) starts right after mm2(b0).
  - PE warmup dummies at t=0 to exit the cold 1.2GHz p-state.

Math folding (host-side, weight-sized only):
  LN:  y_ln = (y - mu_t) * A_t * ln_g + ln_b
  mm1: h_pre = w1p @ ((y - mu)*A) + b1p, w1p = w1*ln_g, b1p = b1 + w1@ln_b
  GRN: h' = h * ss[i] + grn_b[i],  ss = 1 + grn_g * gx * d,
       gx = sqrt(sum_t h^2), d = 1/(mean_i gx + eps)
  mm2: out = (w2 * ss) @ h + (b2 + w2 @ grn_b) + residual
"""
import sys

sys.path.insert(0, "/opt/trn_rl_repo")

import numpy as np
import concourse.bacc as bacc
import concourse.tile as tile
from concourse import mybir
from concourse.bass_utils import run_bass_kernel_spmd

F32 = mybir.dt.float32
F32R = mybir.dt.float32r
F16 = mybir.dt.float16
AF = mybir.ActivationFunctionType
OP = mybir.AluOpType

B, C, T, I, K = 16, 512, 2048, 1536, 7
NCORES = 8
BPC = B // NCORES          # batches per core
CC = C // 128              # 4 c-chunks
IC = I // 128              # 12 i-chunks
TC = T // 512              # 4 t-chunks
TN = 512                   # matmul free-dim tile
LN_EPS = 1e-6
GRN_EPS = 1e-6

_CACHE = {}


def _build(trace_sim=False, reps=1):
    nc = bacc.Bacc("TRN2", target_bir_lowering=False, debug=False,
                   num_devices=NCORES)
    dram = {}

    def din(name, shape, dt=F32):
        dram[name] = nc.dram_tensor(name, shape, dt, kind="ExternalInput").ap()
        return dram[name]

    x_d = din("x", [BPC, C, T], F16)                 # per-core batches (f16!)
    w1pT_d = din("w1pT", [C, I], F16)                # (w1*ln_g).T  [c, i]
    b1p_d = din("b1p", [128, IC])                    # col-chunked b1p
    w2T_d = din("w2T", [I, C], F16)                  # w2.T  [i, c]
    b2p_d = din("b2p", [128, CC])                    # col-chunked b2 + w2@grn_b
    grng_d = din("grng", [128, IC])                  # col-chunked grn_g
    dww_d = din("dww", [128, CC, K])                 # depthwise taps per c-chunk
    dwb_d = din("dwb", [128, CC])                    # depthwise bias per c-chunk
    out_d = nc.dram_tensor("out", [BPC, C, T], F32, kind="ExternalOutput").ap()

    with tile.TileContext(nc, trace_sim=trace_sim) as tc:
        for _ in range(reps):
            _kernel_body(nc, tc, x_d, w1pT_d, b1p_d, w2T_d, b2p_d,
                         grng_d, dww_d, dwb_d, out_d)
    nc.compile()
    return nc


def _kernel_body(nc, tc, x_d, w1pT_d, b1p_d, w2T_d, b2p_d,
                 grng_d, dww_d, dwb_d, out_d):
    from contextlib import ExitStack
    ctx = ExitStack()
    with ctx:
        ctx.enter_context(nc.allow_low_precision(
            reason="f16 on-chip compute is intentional; tolerance is 2e-2"))
        singles = ctx.enter_context(tc.tile_pool(name="singles", bufs=1))
        xp = ctx.enter_context(tc.tile_pool(name="xp", bufs=2))
        yp = ctx.enter_context(tc.tile_pool(name="yp", bufs=2))
        pp = ctx.enter_context(tc.tile_pool(name="pp", bufs=2))
        ysqp = ctx.enter_context(tc.tile_pool(name="ysqp", bufs=4))
        rowp = ctx.enter_context(tc.tile_pool(name="rowp", bufs=4))
        abp = ctx.enter_context(tc.tile_pool(name="abp", bufs=2))
        yscp = ctx.enter_context(tc.tile_pool(name="yscp", bufs=2))
        hp = ctx.enter_context(tc.tile_pool(name="hp", bufs=4))
        sqp = ctx.enter_context(tc.tile_pool(name="sqp", bufs=2))
        gxp = ctx.enter_context(tc.tile_pool(name="gxp", bufs=2))
        w2p = ctx.enter_context(tc.tile_pool(name="w2p", bufs=1))
        w2sp = ctx.enter_context(tc.tile_pool(name="w2sp", bufs=1))
        op_ = ctx.enter_context(tc.tile_pool(name="op", bufs=2))
        # PSUM pools: 2 ph + 2 po + 2 stats + 2 (one [128,1024] AB) = 8 banks
        mmps = ctx.enter_context(tc.tile_pool(name="mmps", bufs=2, space="PSUM"))
        pops = ctx.enter_context(tc.tile_pool(name="pops", bufs=2, space="PSUM"))
        smps = ctx.enter_context(tc.tile_pool(name="smps", bufs=2, space="PSUM"))
        abps = ctx.enter_context(tc.tile_pool(name="abps", bufs=2, space="PSUM"))

        # ---- tiny constants (no DMA deps) ----
        onesr_f = singles.tile([1, 128], F32)
        nc.vector.memset(onesr_f[:], 1.0)
        ones_row = singles.tile([1, 128], F16)    # bcast lhsT (K=1, M=128)
        nc.vector.tensor_copy(ones_row[:], onesr_f[:])
        invc_f = singles.tile([128, 1], F32)
        nc.vector.memset(invc_f[:], 1.0 / C)
        ones_col = singles.tile([128, 1], F16)    # stats lhsT: gives mean directly
        nc.vector.tensor_copy(ones_col[:], invc_f[:])
        ones1_f = singles.tile([128, 1], F32)
        nc.vector.memset(ones1_f[:], 1.0)
        ones_col1 = singles.tile([128, 1], F32R)  # gsum lhsT (K=128, M=1)
        nc.vector.tensor_copy(ones_col1[:], ones1_f[:])
        eps_ln = singles.tile([1, 1], F32)
        nc.vector.memset(eps_ln[:], LN_EPS)
        wrow = singles.tile([1, TN], F16)
        nc.vector.memset(wrow[:], 0.0)

        # ---- PE warmup: exit cold p-state while x DMA + conv run ----
        for _ in range(24):
            wps = mmps.tile([128, TN], F32, tag="mm")
            nc.tensor.matmul(wps[:], ones_row[:], wrow[:], start=True, stop=True)

        # ---- constants ----
        dww = singles.tile([128, CC, K], F32)
        nc.gpsimd.dma_start(dww[:], dww_d)
        dwb = singles.tile([128, CC], F32)
        nc.gpsimd.dma_start(dwb[:], dwb_d)
        b1p = singles.tile([128, IC], F32)
        nc.gpsimd.dma_start(b1p[:], b1p_d)
        b2p = singles.tile([128, CC], F32)
        nc.gpsimd.dma_start(b2p[:], b2p_d)
        grng = singles.tile([128, IC], F32)
        nc.gpsimd.dma_start(grng[:], grng_d)
        w1pT = singles.tile([128, CC, I], F16)
        nc.sync.dma_start(w1pT[:], w1pT_d.rearrange("(cc p) i -> p cc i", p=128))
        w2t = w2p.tile([128, IC, C], F16)
        nc.sync.dma_start(w2t[:], w2T_d.rearrange("(ic p) c -> p ic c", p=128))

        xv = x_d.rearrange("b (cc p) t -> b p cc t", p=128)

        # ---- per-batch persistent tiles ----
        x_tiles = []
        for b in range(BPC):
            x_t = xp.tile([128, CC, T], F16, tag="x")
            for ci in range(CC):
                nc.gpsimd.dma_start(x_t[:, ci, :], xv[b, :, ci, :])
            x_tiles.append(x_t)

        y_tiles = {}
        gxparts = {}
        h_tiles = {}

        def emit_conv_slice(b, a0, a1, pool_cc=()):
            """Depthwise conv over [a0, a1): TSP products (4x) + TT adds (2x).
            Chunks in pool_cc run as stt chains on GPSIMD (startup balance)."""
            x_t = x_tiles[b]
            y_t = y_tiles[b]
            n = a1 - a0
            for ci in range(CC):
                acc = y_t[:, ci, a0:a1]
                eng = nc.gpsimd if ci in pool_cc else nc.vector
                if ci in pool_cc:
                    # Pool rejects the two-scalar-op TSP form; add bias last
                    eng.tensor_scalar(acc, x_t[:, ci, a0:a1],
                                      dww[:, ci, 3:4], None, OP.mult)
                else:
                    eng.tensor_scalar(acc, x_t[:, ci, a0:a1],
                                      dww[:, ci, 3:4], dwb[:, ci:ci + 1],
                                      OP.mult, OP.add)
                for k in (0, 1, 2, 4, 5, 6):
                    d = k - 3
                    lo = max(a0, -d)
                    hi = min(a1, T - d)
                    m = hi - lo
                    if ci in pool_cc:
                        eng.scalar_tensor_tensor(
                            acc[:, lo - a0:hi - a0],
                            x_t[:, ci, lo + d:hi + d],
                            dww[:, ci, k:k + 1],
                            acc[:, lo - a0:hi - a0], OP.mult, OP.add)
                        if k == 6:
                            eng.tensor_scalar(acc, acc, dwb[:, ci:ci + 1],
                                              None, OP.add)
                        continue
                    p_k = pp.tile([128, n], F16, tag="p",
                                  padded_shape=[128, 1024])
                    nc.vector.tensor_scalar(p_k[:, 0:m],
                                            x_t[:, ci, lo + d:hi + d],
                                            dww[:, ci, k:k + 1], None,
                                            OP.mult)
                    nc.vector.tensor_add(acc[:, lo - a0:hi - a0],
                                         acc[:, lo - a0:hi - a0],
                                         p_k[:, 0:m])

        ysq_tiles = {}

        def emit_ysq(b, t):
            y_t = y_tiles[b]
            ts_ = slice(t * TN, (t + 1) * TN)
            ysq = ysqp.tile([128, CC, TN], F16, tag="ysq")
            nc.scalar.activation(ysq[:], y_t[:, :, ts_], AF.Square)
            ysq_tiles[(b, t)] = ysq

        def emit_tile_stats(b, t):
            """LN stats + A/B rows + broadcast + ysc for t-tile. Returns ysc."""
            y_t = y_tiles[b]
            ts_ = slice(t * TN, (t + 1) * TN)
            ysq = ysq_tiles.pop((b, t))
            mu_ps = smps.tile([1, TN], F32, tag="sm")
            ey2_ps = smps.tile([1, TN], F32, tag="sm")
            for ci in range(CC):
                nc.tensor.matmul(mu_ps[:], ones_col[:], y_t[:, ci, ts_],
                                 start=(ci == 0), stop=(ci == CC - 1))
                nc.tensor.matmul(ey2_ps[:], ones_col[:], ysq[:, ci, :],
                                 start=(ci == 0), stop=(ci == CC - 1))
            mu_sb = rowp.tile([1, TN], F32, tag="r")
            nc.vector.tensor_copy(mu_sb[:], mu_ps[:])
            msq = rowp.tile([1, TN], F32, tag="r")
            nc.vector.tensor_mul(msq[:], mu_sb[:], mu_sb[:])
            var = rowp.tile([1, TN], F32, tag="r")
            nc.vector.tensor_sub(var[:], ey2_ps[:], msq[:])
            stdv = rowp.tile([1, TN], F32, tag="r")
            nc.scalar.activation(stdv[:], var[:], AF.Sqrt, bias=eps_ln[:])
            ab_row = rowp.tile([1, 2 * TN], F16, tag="ab")
            nc.vector.reciprocal(ab_row[:, 0:TN], stdv[:])
            # B = -mu * A
            nc.vector.scalar_tensor_tensor(ab_row[:, TN:2 * TN], mu_sb[:], -1.0,
                                           ab_row[:, 0:TN], OP.mult, OP.mult)
            # broadcast A,B across partitions (K=1 matmuls)
            a_ps = abps.tile([128, TN], F32, tag="abps")
            b_ps = abps.tile([128, TN], F32, tag="abps")
            nc.tensor.matmul(a_ps[:], ones_row[:], ab_row[:, 0:TN],
                             start=True, stop=True)
            nc.tensor.matmul(b_ps[:], ones_row[:],
                             ab_row[:, TN:2 * TN], start=True, stop=True)
            ab_sb = abp.tile([128, 2 * TN], F16, tag="absb")
            nc.scalar.activation(ab_sb[:, 0:TN], a_ps[:], AF.Identity)
            nc.scalar.activation(ab_sb[:, TN:2 * TN], b_ps[:], AF.Identity)
            # ysc = y*A + B
            ysc = yscp.tile([128, CC, TN], F16, tag="ysc")
            for ci in range(CC):
                nc.vector.tensor_mul(ysc[:, ci, :], y_t[:, ci, ts_],
                                     ab_sb[:, 0:TN])
                nc.vector.tensor_add(ysc[:, ci, :], ysc[:, ci, :],
                                     ab_sb[:, TN:2 * TN])
            return ysc

        def emit_tile_mm1(b, t, ysc):
            gxpart = gxparts[b]
            h_t = hp.tile([128, IC, TN], F16, tag="h")
            h_tiles[(b, t)] = h_t
            for ii in range(IC):
                ph = mmps.tile([128, TN], F32, tag="mm")
                isl = slice(ii * 128, (ii + 1) * 128)
                for ci in range(CC):
                    nc.tensor.matmul(ph[:], w1pT[:, ci, isl],
                                     ysc[:, ci, :],
                                     start=(ci == 0), stop=(ci == CC - 1))
                nc.scalar.activation(h_t[:, ii, :], ph[:], AF.Gelu,
                                     bias=b1p[:, ii:ii + 1])
                sq = sqp.tile([128, TN], F16, tag="sq")
                if ii % 2 == 0:
                    nc.vector.scalar_tensor_tensor(
                        sq[:], h_t[:, ii, :], 1.0, h_t[:, ii, :],
                        OP.bypass, OP.mult,
                        accum_out=gxpart[:, ii, t:t + 1])
                else:
                    nc.scalar.activation(
                        sq[:], h_t[:, ii, :], AF.Square,
                        accum_out=gxpart[:, ii, t:t + 1])

        def emit_grn(b):
            """GRN scale factors -> scaled w2 copy."""
            gxpart = gxparts[b]
            gxsq = gxp.tile([128, IC], F32, tag="gx2")
            nc.vector.tensor_reduce(gxsq[:], gxpart[:],
                                    axis=mybir.AxisListType.X, op=OP.add)
            gx = gxp.tile([128, IC], F32R, tag="gx2")
            nc.scalar.activation(gx[:], gxsq[:], AF.Sqrt)
            gsum = smps.tile([1, IC], F32, tag="sm")
            nc.tensor.matmul(gsum[:], ones_col1[:], gx[:], start=True, stop=True)
            gtot = gxp.tile([1, 1], F32, tag="gx3")
            nc.vector.tensor_reduce(gtot[:], gsum[:], axis=mybir.AxisListType.X,
                                    op=OP.add)
            dinv = gxp.tile([1, 1], F32, tag="gx3")
            nc.vector.tensor_scalar(dinv[:], gtot[:], 1.0 / I, GRN_EPS,
                                    OP.mult, OP.add)
            d_row = gxp.tile([1, 1], F32R, tag="gx3")
            nc.vector.reciprocal(d_row[:], dinv[:])
            dbc = gxp.tile([128, 1], F32, tag="gx4")
            nc.gpsimd.partition_broadcast(dbc[:], d_row[:].bitcast(F32))
            ss = gxp.tile([128, IC], F32, tag="gx4")
            nc.vector.scalar_tensor_tensor(ss[:], gx[:].bitcast(F32), dbc[:],
                                           grng[:], OP.mult, OP.mult)
            nc.vector.tensor_scalar(ss[:], ss[:], 1.0, None, OP.add)
            w2sc = w2sp.tile([128, IC, C], F16, tag="w2sc")
            for ci in range(CC):
                csl = slice(ci * 128, (ci + 1) * 128)
                for ii in range(IC):
                    nc.vector.tensor_scalar(w2sc[:, ii, csl], w2t[:, ii, csl],
                                            ss[:, ii:ii + 1], None, OP.mult)
            return w2sc

        def emit_mm2(b, w2sc, prefetch=None):
            """mm2 + bias + residual + store. prefetch() emitted mid-stream."""
            x_t = x_tiles[b]
            group = 0
            for t in range(TC):
                ts_ = slice(t * TN, (t + 1) * TN)
                h_t = h_tiles[(b, t)]
                for ci in range(CC):
                    csl = slice(ci * 128, (ci + 1) * 128)
                    po = pops.tile([128, TN], F32, tag="po")
                    for ii in range(IC):
                        nc.tensor.matmul(po[:], w2sc[:, ii, csl],
                                         h_t[:, ii, :],
                                         start=(ii == 0), stop=(ii == IC - 1))
                    o_sb = op_.tile([128, TN], F32)
                    nc.scalar.activation(o_sb[:], po[:], AF.Identity,
                                         bias=b2p[:, ci:ci + 1])
                    nc.gpsimd.tensor_add(o_sb[:], o_sb[:], x_t[:, ci, ts_])
                    nc.gpsimd.dma_start(
                        out_d[b, ci * 128:(ci + 1) * 128, ts_], o_sb[:])
                    group += 1
                    if group == 10 and prefetch is not None:
                        prefetch()

        # ================= schedule =================
        # stats chains run one t-tile ahead of mm1 so the LN row-math /
        # broadcast latency hides under the previous tile's matmuls; ysq
        # ops are emitted right after conv so they don't queue behind GELUs.
        CONV_SLICES = [(0, 512), (512, 1024), (1024, 2048)]
        prefetched_ysc = {}

        for b in range(BPC):
            if b not in y_tiles:
                y_tiles[b] = yp.tile([128, CC, T], F16, tag="y", name=f"y{b}")
                gxparts[b] = gxp.tile([128, IC, TC], F32, tag="gxpart",
                                      name=f"gxp{b}")
            if b in prefetched_ysc:
                ysc_prev = prefetched_ysc.pop(b)
            else:
                emit_conv_slice(b, *CONV_SLICES[0])
                emit_ysq(b, 0)
                ysc_prev = emit_tile_stats(b, 0)
                emit_conv_slice(b, *CONV_SLICES[1])
                emit_ysq(b, 1)
                emit_conv_slice(b, *CONV_SLICES[2])
                emit_ysq(b, 2)
                emit_ysq(b, 3)
            for t in range(TC):
                ysc_next = emit_tile_stats(b, t + 1) if t + 1 < TC else None
                emit_tile_mm1(b, t, ysc_prev)
                ysc_prev = ysc_next
            w2sc = emit_grn(b)
            prefetch = None
            if b + 1 < BPC:
                nb = b + 1
                y_tiles[nb] = yp.tile([128, CC, T], F16, tag="y", name=f"y{nb}")
                gxparts[nb] = gxp.tile([128, IC, TC], F32, tag="gxpart",
                                       name=f"gxp{nb}")
                for i, slc in enumerate(CONV_SLICES):
                    emit_conv_slice(nb, *slc)
                    emit_ysq(nb, min(i, 2))
                emit_ysq(nb, 3)

                def prefetch(nb=nb):
                    prefetched_ysc[nb] = emit_tile_stats(nb, 0)
            emit_mm2(b, w2sc, prefetch=prefetch)


def _host_prep(inputs):
    w1 = inputs["w1"].astype(np.float64)
    ln_g = inputs["ln_g"].astype(np.float64)
    ln_b = inputs["ln_b"].astype(np.float64)
    w2 = inputs["w2"].astype(np.float64)
    w1p = w1 * ln_g[None, :]                         # [I, C]
    prep = {
        "w1pT": np.ascontiguousarray(w1p.T).astype(np.float16),
        "b1p": (inputs["b1"].astype(np.float64) + w1 @ ln_b)
               .astype(np.float32).reshape(IC, 128).T.copy(),
        "w2T": np.ascontiguousarray(w2.T).astype(np.float16),
        "b2p": (inputs["b2"].astype(np.float64)
                + w2 @ inputs["grn_b"].astype(np.float64))
               .astype(np.float32).reshape(CC, 128).T.copy(),
        "grng": inputs["grn_g"].reshape(IC, 128).T.copy().astype(np.float32),
        "dww": inputs["dw_w"].reshape(C, K).reshape(CC, 128, K)
               .transpose(1, 0, 2).copy().astype(np.float32),
        "dwb": inputs["dw_b"].reshape(CC, 128).T.copy().astype(np.float32),
    }
    return prep


def run(inputs, trace=False, **kw):
    if "nc" not in _CACHE:
        _CACHE["nc"] = _build()
    nc = _CACHE["nc"]
    prep = _host_prep(inputs)
    x = np.asarray(inputs["x"], dtype=np.float32).astype(np.float16)
    in_maps = []
    for c in range(NCORES):
        m = dict(prep)
        m["x"] = np.ascontiguousarray(x[c * BPC:(c + 1) * BPC])
        in_maps.append(m)
    res = run_bass_kernel_spmd(nc, in_maps, core_ids=list(range(NCORES)),
                               trace=trace, **kw)
    out = np.concatenate([r["out"] for r in res.results], axis=0)
    return out, res


def kernel(**inputs):
    out, _ = run(inputs)
    return out
